# revision 50
# baseline (speedup 1.0000x reference)
"""DrBC GNN message-passing kernel for 8 Trainium2 NeuronCores.

Strategy (graph/data parallel, per sharding hint):
 - Nodes sharded contiguously across 8 cores (6250/core). Edges partitioned
   by destination core so segment_sum is local.
 - Aggregation h_agg^T = sum_e msg[e,:] * onehot(dst_e)*norm_e is computed on
   the TensorEngine: edges are bucketed on the host into (dst-window, src-block)
   chunks of 128; per chunk a [128e x 64d] norm-scaled one-hot tile S^T is
   prebuilt on the host and kept resident in SBUF; PSUM accumulates per window.
 - Messages h[src] are fetched with GPSIMD dma_gather from a bf16 node-major
   table in DRAM (indices are int16, so the 50000-row table is addressed in two
   blocks of <=32768 rows; host sorts edges by block).
 - GRU runs feature-major ([128 feat x 512 nodes] tiles) with the six 128x128
   weight blocks stationary on the PE; gates on the Scalar engine.
 - Per layer the updated bf16 h shard is transposed (PE) to node-major rows and
   AllGathered so every core has the full gather table.
 - BatchNorm (training mode, global over nodes) uses a tiny AllReduce of
   per-core (sum, sumsq).
"""

import os
import sys
from dataclasses import dataclass, field

import numpy as np

sys.path.insert(0, "/opt/trn_rl_repo")

import concourse.bass as bass  # noqa: E402
import concourse.bacc as bacc  # noqa: E402
import concourse.tile as tile  # noqa: E402
from concourse import mybir  # noqa: E402
from concourse import bass_utils  # noqa: E402
from concourse.masks import make_identity  # noqa: E402
from concourse.tile import add_dep_helper  # noqa: E402
from concourse.ap import AP as RawAP  # noqa: E402
from ml_dtypes import bfloat16  # noqa: E402

FP = mybir.dt.float32
BF = mybir.dt.bfloat16
I16 = mybir.dt.int16
AX = mybir.AxisListType
ALU = mybir.AluOpType
ACT = mybir.ActivationFunctionType


@dataclass
class Cfg:
    N: int = 50000
    D: int = 128
    L: int = 5
    NCORES: int = 8
    W: int = 128         # dst window width (= agg matmul free dim)
    GW: int = 4          # windows per group (group = GRU node tile of GW*W)
    BLK: int = 32768     # int16 gather index block size
    EPS: float = 1e-5
    msg_bufs: int = 2
    NQ: int = 4
    QSPLIT: int = 0  # >0: N-way call split; <=0: single-packet calls of <=7 chunks
    MAXSP: int = 7   # chunks per single-packet gather call
    CCEND: bool = False  # emit both table CCs at layer end
    LREP: int = 1  # timing amplifier: repeat the layer loop
    F32: bool = False  # full-fp32 pipeline (gather is descriptor-bound, so ~free)
    SEG: int = 4096      # rows/core in table segment A (8*SEG == BLK)
    CCSPLIT: bool = True  # two overlapped per-segment AllGathers per layer
    cc_mini: bool = False  # timing probe: tiny CCs in place of table CCs
    TDIRECT: bool = False  # direct table copy: data not visible cross-core
    GIND: bool = False   # gather via indirect_dma_start: ~8x slower, dead end
    # ablation switches (timing experiments only; results become wrong)
    abl_cc: bool = False
    abl_gather: bool = False
    abl_slab: bool = False
    abl_aggmm: bool = False
    abl_gru: bool = False
    abl_wt: bool = False

    @property
    def NL(self):
        return self.N // self.NCORES

    @property
    def NWIN(self):
        return -(-self.NL // self.W)

    @property
    def NLP(self):
        return self.NWIN * self.W

    @property
    def NG(self):
        return -(-self.NWIN // self.GW)


@dataclass
class Plan:
    cfg: Cfg
    nch: np.ndarray          # [NWIN, 2] chunks per (window, block), uniform across cores
    order: list              # list of (w, b) in stream order
    chunk_base: dict         # (w, b) -> first chunk index
    TOTCH: int = 0
    calls: list = field(default_factory=list)   # [g][b] = (ch0, ncc)
    MAXC: int = 0
    sched: list = field(default_factory=list)   # [g] = list of (ch, wl, first, last)
    in_maps: list = field(default_factory=list)


def _win_range(cfg, g):
    w0 = g * cfg.GW
    w1 = min((g + 1) * cfg.GW, cfg.NWIN)
    return range(w0, w1)


def make_plan(cfg: Cfg, edge_index, norm, x, W_enc, bn_e_gamma, bn_e_beta,
              W_ih, W_hh, b_ih, b_hh, W_dec, bn_d_gamma, bn_d_beta, W_dec2):
    src = np.asarray(edge_index[0], dtype=np.int64)
    dst = np.asarray(edge_index[1], dtype=np.int64)
    norm = np.asarray(norm, dtype=np.float32)
    NL, W, NWIN, NCORES = cfg.NL, cfg.W, cfg.NWIN, cfg.NCORES

    core_of = dst // NL
    percore = []
    cnt = np.zeros((NCORES, NWIN, 2), dtype=np.int64)
    for c in range(NCORES):
        sel = core_of == c
        s, dl, nm = src[sel], dst[sel] - c * NL, norm[sel]
        w = dl // W
        if cfg.CCSPLIT and not cfg.GIND:
            # two-segment table layout: tfull = [all cores' rows 0:SEG |
            # all cores' rows SEG:NL]; segment boundary == BLK so the int16
            # b-split coincides with the two AllGather segments.
            assert NCORES * cfg.SEG == cfg.BLK
            sc_ = s // NL
            sr = s % NL
            s = np.where(sr < cfg.SEG, sc_ * cfg.SEG + sr,
                         NCORES * cfg.SEG + sc_ * (NL - cfg.SEG)
                         + (sr - cfg.SEG))
        if cfg.GIND:
            b = np.zeros_like(s)  # int32 indices: one view, no block split
        else:
            b = (s >= cfg.BLK).astype(np.int64)
        np.add.at(cnt[c], (w, b), 1)
        percore.append((s, dl, nm, w, b))

    nch = -(-cnt.max(axis=0) // 128)             # [NWIN, 2]
    nch[:, 0] = np.maximum(nch[:, 0], 1)         # every window has >=1 chunk

    order, chunk_base = [], {}
    acc = 0
    calls = []
    for g in range(cfg.NG):
        calls.append([])
        for b in (0, 1):
            ch0 = acc
            for w in _win_range(cfg, g):
                order.append((w, b))
                chunk_base[(w, b)] = acc
                acc += int(nch[w, b])
            calls[g].append((ch0, acc - ch0))
    TOTCH = acc
    MAXC = max(ncc for g in calls for (_, ncc) in g if ncc > 0)

    # matmul schedule per group: (chunk, window-local, first, last)
    sched = []
    for g in range(cfg.NG):
        items = []
        for b in (0, 1):
            for w in _win_range(cfg, g):
                base = chunk_base[(w, b)]
                for j in range(int(nch[w, b])):
                    items.append([base + j, w - g * cfg.GW, False, False])
        # one accumulation group per PSUM bank: start on globally-first matmul,
        # stop on globally-last (first touch of each element overwrites via
        # the pending-zero / has_written mechanism).
        items[0][2] = True
        items[-1][3] = True
        sched.append([tuple(it) for it in items])

    plan = Plan(cfg=cfg, nch=nch, order=order, chunk_base=chunk_base,
                TOTCH=TOTCH, calls=calls, MAXC=MAXC, sched=sched)

    # ---- per-core input tensors ----
    xdt = np.float32 if cfg.F32 else bfloat16
    D = cfg.D
    W_ih = np.asarray(W_ih, np.float32)
    W_hh = np.asarray(W_hh, np.float32)
    b_ih = np.asarray(b_ih, np.float32)
    b_hh = np.asarray(b_hh, np.float32)
    gru = np.concatenate([
        W_ih[0:D].T, W_ih[D:2 * D].T, W_ih[2 * D:3 * D].T,
        W_hh[0:D].T, W_hh[D:2 * D].T, W_hh[2 * D:3 * D].T,
    ], axis=1).astype(xdt)                                # [128, 6*128]
    gbias = np.stack([
        b_ih[0:D] + b_hh[0:D], b_ih[D:2 * D] + b_hh[D:2 * D],
        b_ih[2 * D:3 * D], b_hh[2 * D:3 * D],
    ], axis=1).astype(np.float32)                          # [128, 4]
    wenc = np.ascontiguousarray(np.asarray(W_enc, np.float32).T)   # [3,128]
    # encode BN stats are data-independent of device state: e0 = x @ W_enc.T
    e0 = np.asarray(x, np.float64) @ np.asarray(W_enc, np.float64).T
    mu_e = e0.mean(0)
    var_e = e0.var(0)
    sc = np.asarray(bn_e_gamma, np.float64) / np.sqrt(var_e + cfg.EPS)
    sh = np.asarray(bn_e_beta, np.float64) - mu_e * sc
    bne = np.stack([sc, sh], axis=1).astype(np.float32)            # [128,2]
    wdec = np.ascontiguousarray(np.asarray(W_dec, np.float32).T).astype(xdt)  # [128,64]
    bnd = np.stack([np.asarray(bn_d_gamma, np.float32),
                    np.asarray(bn_d_beta, np.float32)], axis=1)    # [64,2]
    wdec2 = np.ascontiguousarray(np.asarray(W_dec2, np.float32).T).astype(xdt)  # [64,1]

    x = np.asarray(x, np.float32)
    for c in range(NCORES):
        s, dl, nm, w, b = percore[c]
        idx_arr = np.zeros(TOTCH * 128, np.int32 if cfg.GIND else np.int16)
        sval = np.zeros(TOTCH * 128, np.float32)
        dcol = np.zeros(TOTCH * 128, np.int64)
        # stable bucket fill
        key = w * 2 + b
        ordk = np.argsort(key, kind="stable")
        s, dl, nm, w, b = s[ordk], dl[ordk], nm[ordk], w[ordk], b[ordk]
        ks = key[ordk]
        if len(ks):
            change = np.r_[True, ks[1:] != ks[:-1]]
            run_start = np.flatnonzero(change)
            run_id = np.cumsum(change) - 1
            pos = np.arange(len(s)) - run_start[run_id]
        else:
            pos = np.zeros(0, np.int64)
        cb_lut = np.zeros(NWIN * 2, np.int64)
        for (wi, bi), v in chunk_base.items():
            cb_lut[wi * 2 + bi] = v
        slot = cb_lut[ks] * 128 + pos
        if cfg.GIND:
            idx_arr[slot] = s.astype(np.int32)
        else:
            idx_arr[slot] = (s - b * cfg.BLK).astype(np.int16)
        sval[slot] = nm
        dcol[slot] = dl - w * cfg.W

        S = np.zeros((TOTCH, 128, cfg.W), np.float32)
        ii = np.arange(TOTCH * 128)
        S[ii // 128, ii % 128, dcol] = sval
        stab = np.ascontiguousarray(
            S.transpose(1, 0, 2).reshape(128, TOTCH * cfg.W)).astype(xdt)
        if cfg.GIND:
            # [128, TOTCH]: column ch, partition p = edge slot ch*128+p
            idx_wrapped = np.ascontiguousarray(idx_arr.reshape(TOTCH, 128).T)
        else:
            idx_wrapped = np.ascontiguousarray(np.tile(idx_arr.reshape(TOTCH * 8, 16).T, (8, 1)))

        xT = np.zeros((3, cfg.NLP), np.float32)
        xT[:, :NL] = x[c * NL:(c + 1) * NL].T

        plan.in_maps.append({
            "xT": xT, "idx": idx_wrapped, "stab": stab,
            "wenc": wenc, "gru": gru, "gbias": gbias, "bne": bne,
            "wdec": wdec, "bnd": bnd, "wdec2": wdec2,
        })
    return plan


def build_program(plan: Plan, num_devices: int):
    cfg = plan.cfg
    N, D, NL, NLP, W, GW, NG, L = (cfg.N, cfg.D, cfg.NL, cfg.NLP, cfg.W,
                                   cfg.GW, cfg.NG, cfg.L)
    TOTCH, MAXC = plan.TOTCH, plan.MAXC
    RG = [list(range(num_devices))]

    XD = FP if cfg.F32 else BF
    nc = bacc.Bacc("TRN2", target_bir_lowering=False, debug=False,
                   num_devices=num_devices, num_swdge_queues=cfg.NQ)

    # ---- external tensors ----
    t_xT = nc.dram_tensor("xT", [3, NLP], FP, kind="ExternalInput").ap()
    if cfg.GIND:
        t_idx = nc.dram_tensor("idx", [128, TOTCH], mybir.dt.int32,
                               kind="ExternalInput").ap()
    else:
        t_idx = nc.dram_tensor("idx", [128, TOTCH * 8], I16,
                               kind="ExternalInput").ap()
    t_stab = nc.dram_tensor("stab", [128, TOTCH * W], XD, kind="ExternalInput").ap()
    t_wenc = nc.dram_tensor("wenc", [3, 128], FP, kind="ExternalInput").ap()
    t_gru = nc.dram_tensor("gru", [128, 6 * 128], XD, kind="ExternalInput").ap()
    t_gbias = nc.dram_tensor("gbias", [128, 4], FP, kind="ExternalInput").ap()
    t_bne = nc.dram_tensor("bne", [128, 2], FP, kind="ExternalInput").ap()
    t_wdec = nc.dram_tensor("wdec", [128, 64], XD, kind="ExternalInput").ap()
    t_bnd = nc.dram_tensor("bnd", [64, 2], FP, kind="ExternalInput").ap()
    t_wdec2 = nc.dram_tensor("wdec2", [64, 1], XD, kind="ExternalInput").ap()
    t_y = nc.dram_tensor("y", [NL], FP, kind="ExternalOutput").ap()

    GN = GW * W

    def gwidth(g):
        return min(GN, NLP - g * GN)

    def nodes_slice(g):
        return slice(g * GN, g * GN + gwidth(g))

    if cfg.TDIRECT:
        assert cfg.CCSPLIT
    assert not (cfg.GIND and cfg.CCSPLIT), "GIND needs the single-table view"
    with tile.TileContext(nc) as tc:
        with (
            tc.tile_pool(name="const", bufs=1) as cpool,
            tc.tile_pool(name="state", bufs=1) as spool,
            tc.tile_pool(name="msg", bufs=cfg.msg_bufs) as mpool,
            tc.tile_pool(name="work", bufs=2) as wpool,
            tc.tile_pool(name="psA", bufs=2, space="PSUM") as psA,
            tc.tile_pool(name="psG", bufs=1, space="PSUM") as psG,
            tc.tile_pool(name="psT", bufs=2, space="PSUM") as psT,
            tc.tile_pool(name="dram", bufs=1, space="DRAM") as dpool,
        ):
            # ---------- load constants ----------
            if cfg.GIND:
                idx_sb = cpool.tile([128, TOTCH], mybir.dt.int32)
            else:
                idx_sb = cpool.tile([128, TOTCH * 8], I16)
            nc.sync.dma_start(idx_sb[:], t_idx)
            wenc_sb = cpool.tile([3, 128], FP)
            nc.sync.dma_start(wenc_sb[:], t_wenc)
            gru_sb = cpool.tile([128, 6 * 128], XD)
            nc.sync.dma_start(gru_sb[:], t_gru)
            gbias_sb = cpool.tile([128, 4], FP)
            nc.sync.dma_start(gbias_sb[:], t_gbias)
            bne_sb = cpool.tile([128, 2], FP)
            nc.sync.dma_start(bne_sb[:], t_bne)
            wdec_sb = cpool.tile([128, 64], XD)
            nc.sync.dma_start(wdec_sb[:], t_wdec)
            bnd_sb = cpool.tile([64, 2], FP)
            nc.sync.dma_start(bnd_sb[:], t_bnd)
            wdec2_sb = cpool.tile([64, 1], XD)
            nc.sync.dma_start(wdec2_sb[:], t_wdec2)
            ident_bf = cpool.tile([128, 128], XD)
            make_identity(nc, ident_bf[:])

            def gw(i):  # gru weight block i as lhsT [128,128]
                return gru_sb[:, i * 128:(i + 1) * 128]

            # ---------- state ----------
            hsum = spool.tile([128, NLP], FP)
            prevh = spool.tile([128, NLP], XD)

            # internal DRAM
            TL = L * cfg.LREP
            tin = [dpool.tile([NL, D], XD, name=f"tin{i}") for i in range(TL)]
            if cfg.CCSPLIT:
                # one tensor per AllGather segment: keeps the sim's
                # single-writer check happy and gives exact read deps
                tfullA = [dpool.tile([cfg.BLK, D], XD, addr_space="Shared",
                                     name=f"tfA{i}") for i in range(TL)]
                tfullB = [dpool.tile([N - cfg.BLK, D], XD, addr_space="Shared",
                                     name=f"tfB{i}") for i in range(TL)]
            else:
                tfull = [dpool.tile([N, D], XD, addr_space="Shared",
                                    name=f"tfull{i}") for i in range(TL)]
            bar_i = dpool.tile([1, D], FP, name="bar_i")
            nc.sync.dma_start(bar_i[:, :], t_xT[0:1, 0:D])
            stats_io = [dpool.tile([128, 2], FP, addr_space=("Shared" if i else "Local"),
                                   name=f"stio{i}") for i in range(2)]
            stats2_io = [dpool.tile([64, 2], FP, addr_space=("Shared" if i else "Local"),
                                    name=f"st2io{i}") for i in range(2)]

            # small persistent tiles
            acc_s = spool.tile([128, NG], FP)
            acc_q = spool.tile([128, NG], FP)
            accd_s = spool.tile([64, NG], FP)
            accd_q = spool.tile([64, NG], FP)
            stats_sb = spool.tile([128, 2], FP)
            gstats_sb = spool.tile([128, 2], FP)
            scale_e = spool.tile([128, 1], FP)
            shift_e = spool.tile([128, 1], FP)
            stats2_sb = spool.tile([64, 2], FP)
            gstats2_sb = spool.tile([64, 2], FP)
            scale_d = spool.tile([64, 1], FP)
            shift_d = spool.tile([64, 1], FP)
            t_y2d = t_y.rearrange("(a b) -> a b", a=1)

            def bn_coeffs(gst, gamma_beta, scale_t, shift_t, nelem, P):
                """scale/shift from global (sum,sumsq) in gst [P,2]."""
                mu = wpool.tile([128, 1], FP, tag="bn1")
                ex2 = wpool.tile([128, 1], FP, tag="bn2")
                var = wpool.tile([128, 1], FP, tag="bn3")
                std = wpool.tile([128, 1], FP, tag="bn4")
                nc.vector.tensor_scalar_mul(mu[:P], gst[:, 0:1], 1.0 / nelem)
                nc.vector.tensor_scalar_mul(ex2[:P], gst[:, 1:2], 1.0 / nelem)
                nc.vector.tensor_tensor(var[:P], mu[:P], mu[:P], op=ALU.mult)
                nc.vector.tensor_tensor(var[:P], ex2[:P], var[:P], op=ALU.subtract)
                nc.vector.tensor_scalar_add(var[:P], var[:P], cfg.EPS)
                nc.scalar.activation(std[:P], var[:P], ACT.Sqrt)
                nc.vector.reciprocal(std[:P], std[:P])
                nc.vector.tensor_tensor(scale_t[:], std[:P], gamma_beta[:, 0:1],
                                        op=ALU.mult)
                nc.vector.tensor_tensor(var[:P], mu[:P], scale_t[:], op=ALU.mult)
                nc.vector.tensor_tensor(shift_t[:], gamma_beta[:, 1:2], var[:P],
                                        op=ALU.subtract)

            def allreduce_stats(sb, io_pair, P):
                nc.sync.dma_start(io_pair[0][:], sb[:P, :])
                nc.gpsimd.collective_compute(
                    "AllReduce", ALU.add, replica_groups=RG,
                    ins=[io_pair[0].opt()], outs=[io_pair[1].opt()])
                return io_pair[1]

            # ================= ENCODE =================
            scratch = wpool.tile([128, GN], FP, tag="scr")

            def load_xt(g):
                wd = gwidth(g)
                xt = wpool.tile([3, GN], FP, tag="xt")
                nc.sync.dma_start(xt[:, :wd], t_xT[:, nodes_slice(g)])
                return xt

            def write_table(l, g):
                """transpose prevh group g to node-major rows of tin[l%2]."""
                wd = gwidth(g)
                for k in range(wd // 128 + (1 if wd % 128 else 0)):
                    n0 = g * GN + k * 128
                    rows = min(128, NL - n0)
                    if rows <= 0:
                        continue
                    pst = psT.tile([128, 128], XD, tag="tr")
                    nc.tensor.transpose(pst[:], prevh[:, n0:n0 + 128], ident_bf[:])
                    tt = wpool.tile([128, 128], XD, tag="tt")
                    nc.scalar.activation(tt[:], pst[:], ACT.Copy)
                    nc.sync.dma_start(tin[l][n0:n0 + rows, :], tt[:rows, :])

            barrier = {}

            def emit_cc(l, seg):
                """AllGather one table segment (two-segment shared layout)."""
                if cfg.abl_cc:
                    return
                if cfg.TDIRECT:
                    # each core DMA-copies its shard into the shared segment
                    # at a pid-dependent offset (single writer instruction per
                    # tensor), then a tiny AllGather acts as the cross-core
                    # barrier; gathers take a manual dep on it.
                    if seg == 0:
                        rows, s0, tf = cfg.SEG, 0, tfullA[l]
                    else:
                        rows, s0, tf = NL - cfg.SEG, cfg.SEG, tfullB[l]
                    dsl = tf[0:rows, :]
                    pid = nc.sync.partition_id()
                    dst = RawAP(tensor=dsl.tensor, offset=pid * (rows * D),
                                ap=dsl.ap, dep_tracking_offset=0)
                    cp = nc.sync.dma_start(dst, tin[l][s0:s0 + rows, :])
                    bar_o = dpool.tile([num_devices, D], FP,
                                       addr_space="Shared",
                                       name=f"bo{l}_{seg}")
                    cc = nc.gpsimd.collective_compute(
                        "AllGather", ALU.bypass, replica_groups=RG,
                        ins=[bar_i.opt()], outs=[bar_o.opt()])
                    add_dep_helper(cc.ins, cp.ins, reason="barrier after table copy")
                    barrier[(l, seg)] = cc
                    return
                if cfg.cc_mini:
                    # tiny CC in the same dependency position (timing probe):
                    # in <- one tin row (orders after table writes), out -> one
                    # row of the segment (orders gathers after the CC)
                    s0 = 0 if seg == 0 else cfg.SEG
                    tf = tfullA[l] if seg == 0 else tfullB[l]
                    mbar_i = dpool.tile([1, D], XD, name=f"bi{l}_{seg}")
                    bar_o = dpool.tile([num_devices, D], XD,
                                       addr_space="Shared",
                                       name=f"bo{l}_{seg}")
                    bsb = wpool.tile([1, D], XD, tag="barb")
                    nc.sync.dma_start(bsb[:, :], tin[l][s0:s0 + 1, :])
                    nc.sync.dma_start(mbar_i[:, :], bsb[:, :])
                    nc.gpsimd.collective_compute(
                        "AllGather", ALU.bypass, replica_groups=RG,
                        ins=[mbar_i.opt()], outs=[bar_o.opt()])
                    nc.sync.dma_start(tf[0:1, :], bar_o[0:1, :])
                    return
                if seg == 0:
                    nc.gpsimd.collective_compute(
                        "AllGather", ALU.bypass, replica_groups=RG,
                        ins=[tin[l][0:cfg.SEG, :].opt()],
                        outs=[tfullA[l].opt()])
                else:
                    nc.gpsimd.collective_compute(
                        "AllGather", ALU.bypass, replica_groups=RG,
                        ins=[tin[l][cfg.SEG:NL, :].opt()],
                        outs=[tfullB[l].opt()])

            for g in range(NG):
                wd = gwidth(g)
                sl = nodes_slice(g)
                xt = load_xt(g)
                ps = psA.tile([128, GN], FP, tag="agg")
                nc.tensor.matmul(ps[:, :wd], wenc_sb[:], xt[:, :wd],
                                 start=True, stop=True)
                nc.scalar.activation(prevh[:, sl], ps[:, :wd], ACT.Relu,
                                     bias=bne_sb[:, 1:2], scale=bne_sb[:, 0:1])
                nc.scalar.activation(hsum[:, sl], ps[:, :wd], ACT.Relu,
                                     bias=bne_sb[:, 1:2], scale=bne_sb[:, 0:1])
                if not cfg.abl_wt:
                    write_table(0, g)
                if cfg.CCSPLIT and g == cfg.SEG // GN - 1:
                    emit_cc(0, 0)
            if cfg.CCSPLIT:
                emit_cc(0, 1)

            scratch2 = wpool.tile([64, GN], FP, tag="scr2")
            # ================= LAYERS =================
            # queue must track Tile's DMASW lane round-robin (lane = k % 8,
            # queue = k % NQ with NQ | 8) -> single global SWDGE-DMA counter.
            qctr = 0
            for l in range(TL):
                last = l == TL - 1
                if not cfg.CCSPLIT and not cfg.abl_cc:
                    nc.gpsimd.collective_compute(
                        "AllGather", ALU.bypass, replica_groups=RG,
                        ins=[tin[l].opt()], outs=[tfull[l].opt()])
                if cfg.CCSPLIT:
                    tblA, tblB = tfullA[l][:, :], tfullB[l][:, :]
                else:
                    tblA, tblB = tfull[l][:, :], tfull[l][cfg.BLK:, :]
                for g in range(NG):
                    wd = gwidth(g)
                    sl = nodes_slice(g)
                    ps = psA.tile([128, GN], FP, tag="agg")
                    msgs = {}
                    for b in (0, 1):
                        ch0, ncc = plan.calls[g][b]
                        if ncc == 0:
                            continue
                        m = mpool.tile([128, MAXC, 128], XD, tag="msg")
                        if cfg.abl_gather:
                            # keep the tile allocated (timing ablation only)
                            nc.sync.dma_start(m[:, 0, :], t_stab[:, 0:128])
                        src_view = tblB if b else tblA
                        if cfg.GIND:
                            if not cfg.abl_gather:
                                nc.gpsimd.indirect_dma_start(
                                    out=m[:, 0:ncc, :], out_offset=None,
                                    in_=src_view,
                                    in_offset=bass.IndirectOffsetOnAxis(
                                        ap=idx_sb[:, ch0:ch0 + ncc], axis=0))
                        else:
                            if cfg.QSPLIT > 0:
                                nsplit = min(cfg.QSPLIT, ncc)
                                sp = False
                            else:  # QSPLIT<=0: single-packet calls
                                nsplit = -(-ncc // cfg.MAXSP)
                                sp = True
                            per = -(-ncc // nsplit)
                            for si in range(nsplit):
                                c0 = si * per
                                c1 = min(ncc, c0 + per)
                                if c1 <= c0:
                                    continue
                                if not cfg.abl_gather:
                                    gi = nc.gpsimd.dma_gather(
                                        m[:, c0:c1, :], src_view,
                                        idx_sb[:, (ch0 + c0) * 8:(ch0 + c1) * 8],
                                        (c1 - c0) * 128, (c1 - c0) * 128, 128,
                                        elem_step=D, single_packet=sp,
                                        queue_num=qctr % cfg.NQ)
                                    qctr += 1
                                    bb = barrier.get((l, b))
                                    if bb is not None:
                                        add_dep_helper(
                                            gi.ins, bb.ins,
                                            reason="gather after table barrier")
                        slab = mpool.tile([128, MAXC * W], XD, tag="slab")
                        if cfg.abl_slab:
                            nc.sync.dma_start(slab[:, 0:W], t_stab[:, 0:W])
                        else:
                            nc.sync.dma_start(slab[:, :ncc * W],
                                              t_stab[:, ch0 * W:(ch0 + ncc) * W])
                        msgs[b] = (m, slab, ch0)
                    if (cfg.CCSPLIT and not cfg.CCEND and not last
                            and g == cfg.SEG // GN):
                        emit_cc(l + 1, 0)
                    b1_start = plan.calls[g][1][0]
                    b1_n = plan.calls[g][1][1]
                    if cfg.abl_aggmm:
                        m, slab, ch0 = msgs[0]
                        for wl in range(-(-wd // W)):
                            nc.tensor.matmul(
                                ps[:, wl * W:(wl + 1) * W], m[:, 0, :],
                                slab[:, 0:W], start=True, stop=True,
                                skip_group_check=True)
                    else:
                        for (ch, wl, first, lastc) in plan.sched[g]:
                            b = 1 if (b1_n > 0 and ch >= b1_start) else 0
                            m, slab, ch0 = msgs[b]
                            nc.tensor.matmul(
                                ps[:, wl * W:(wl + 1) * W], m[:, ch - ch0, :],
                                slab[:, (ch - ch0) * W:(ch - ch0 + 1) * W],
                                start=first, stop=lastc, skip_group_check=True)
                    # h_sum += h_agg
                    nc.vector.tensor_tensor(hsum[:, sl], hsum[:, sl], ps[:, :wd],
                                            op=ALU.add)
                    if last:
                        # fused decode stats (pass 1) for this group
                        lim = min(wd, max(0, NL - g * GN))
                        zb = wpool.tile([128, GN], XD, tag="zb")
                        nc.scalar.activation(zb[:, :wd], hsum[:, sl], ACT.Copy,
                                             scale=1.0 / (L + 1))
                        ps_y = psG.tile([128, GN], FP, tag="ps_r")
                        nc.tensor.matmul(ps_y[:64, :wd], wdec_sb[:], zb[:, :wd],
                                         start=True, stop=True)
                        nc.scalar.activation(scratch2[:, :lim], ps_y[:64, :lim],
                                             ACT.Copy, accum_out=accd_s[:, g:g + 1])
                        nc.scalar.activation(scratch2[:, :lim], ps_y[:64, :lim],
                                             ACT.Square, accum_out=accd_q[:, g:g + 1])
                        continue
                    if cfg.abl_gru:
                        if not cfg.abl_wt:
                            write_table(l + 1, g)
                        continue
                    aggbf = wpool.tile([128, GN], XD, tag="aggbf")
                    nc.scalar.activation(aggbf[:, :wd], ps[:, :wd], ACT.Copy)
                    # GRU
                    ph = prevh[:, sl]
                    ab = aggbf[:, :wd]
                    ps_r = psG.tile([128, GN], FP, tag="ps_r")
                    ps_z = psG.tile([128, GN], FP, tag="ps_z")
                    ps_i = psG.tile([128, GN], FP, tag="ps_i")
                    ps_h = psG.tile([128, GN], FP, tag="ps_h")
                    nc.tensor.matmul(ps_r[:, :wd], gw(0), ab, start=True, stop=False)
                    nc.tensor.matmul(ps_r[:, :wd], gw(3), ph, start=False, stop=True)
                    nc.tensor.matmul(ps_z[:, :wd], gw(1), ab, start=True, stop=False)
                    nc.tensor.matmul(ps_z[:, :wd], gw(4), ph, start=False, stop=True)
                    nc.tensor.matmul(ps_i[:, :wd], gw(2), ab, start=True, stop=True)
                    nc.tensor.matmul(ps_h[:, :wd], gw(5), ph, start=True, stop=True)
                    r_t = wpool.tile([128, GN], XD, tag="r_t")
                    z_t = wpool.tile([128, GN], XD, tag="z_t")
                    hn_t = wpool.tile([128, GN], XD, tag="hn_t")
                    t2 = wpool.tile([128, GN], FP, tag="t2")
                    n_t = wpool.tile([128, GN], XD, tag="n_t")
                    d_t = wpool.tile([128, GN], XD, tag="d_t")
                    nc.scalar.activation(r_t[:, :wd], ps_r[:, :wd], ACT.Sigmoid,
                                         bias=gbias_sb[:, 0:1])
                    nc.scalar.activation(z_t[:, :wd], ps_z[:, :wd], ACT.Sigmoid,
                                         bias=gbias_sb[:, 1:2])
                    nc.scalar.activation(hn_t[:, :wd], ps_h[:, :wd], ACT.Identity,
                                         bias=gbias_sb[:, 3:4])
                    nc.vector.tensor_tensor(t2[:, :wd], r_t[:, :wd], hn_t[:, :wd],
                                            op=ALU.mult)
                    nc.vector.tensor_tensor(t2[:, :wd], t2[:, :wd], ps_i[:, :wd],
                                            op=ALU.add)
                    nc.scalar.activation(n_t[:, :wd], t2[:, :wd], ACT.Tanh,
                                         bias=gbias_sb[:, 2:3])
                    nc.vector.tensor_tensor(d_t[:, :wd], ph, n_t[:, :wd],
                                            op=ALU.subtract)
                    nc.vector.tensor_tensor(d_t[:, :wd], z_t[:, :wd], d_t[:, :wd],
                                            op=ALU.mult)
                    nc.vector.tensor_tensor(ph, n_t[:, :wd], d_t[:, :wd],
                                            op=ALU.add)
                    if not cfg.abl_wt:
                        write_table(l + 1, g)
                if cfg.CCSPLIT and not last:
                    if cfg.CCEND:
                        emit_cc(l + 1, 0)
                    emit_cc(l + 1, 1)

            # ================= DECODE =================
            inv6 = 1.0 / (L + 1)
            nc.vector.reduce_sum(stats2_sb[:, 0:1], accd_s[:], axis=AX.X)
            nc.vector.reduce_sum(stats2_sb[:, 1:2], accd_q[:], axis=AX.X)
            gst2 = allreduce_stats(stats2_sb, stats2_io, 64)
            nc.sync.dma_start(gstats2_sb[:], gst2[:])
            bn_coeffs(gstats2_sb, bnd_sb, scale_d, shift_d, N, 64)

            for g in range(NG):
                wd = gwidth(g)
                sl = nodes_slice(g)
                zb = wpool.tile([128, GN], XD, tag="zb")
                nc.scalar.activation(zb[:, :wd], hsum[:, sl], ACT.Copy, scale=inv6)
                ps = psA.tile([128, GN], FP, tag="agg")
                nc.tensor.matmul(ps[:64, :wd], wdec_sb[:], zb[:, :wd],
                                 start=True, stop=True)
                y2 = wpool.tile([64, GN], XD, tag="y2")
                nc.scalar.activation(y2[:, :wd], ps[:64, :wd], ACT.Relu,
                                     bias=shift_d[:], scale=scale_d[:])
                psf = psA.tile([1, GN], FP, tag="agg")
                nc.tensor.matmul(psf[:, :wd], wdec2_sb[:], y2[:, :wd],
                                 start=True, stop=True)
                lim = min(wd, max(0, NL - g * GN))
                yt = wpool.tile([1, GN], FP, tag="yt")
                nc.vector.tensor_copy(yt[:, :wd], psf[:, :wd])
                nc.sync.dma_start(t_y2d[0:1, g * GN:g * GN + lim], yt[0:1, :lim])

    nc.compile()
    return nc


_CACHE = {}


def _get_program(plan, num_devices):
    key = (plan.TOTCH, plan.MAXC, repr(plan.cfg))
    if key not in _CACHE:
        _CACHE[key] = build_program(plan, num_devices)
    return _CACHE[key]


def kernel(x, edge_index, norm, W_enc, bn_e_gamma, bn_e_beta, W_ih, W_hh,
           b_ih, b_hh, W_dec, bn_d_gamma, bn_d_beta, W_dec2, _trace=False):
    cfg = Cfg()
    plan = make_plan(cfg, edge_index, norm, x, W_enc, bn_e_gamma, bn_e_beta,
                     W_ih, W_hh, b_ih, b_hh, W_dec, bn_d_gamma, bn_d_beta, W_dec2)
    nc = _get_program(plan, cfg.NCORES)
    res = bass_utils.run_bass_kernel_spmd(
        nc, plan.in_maps, core_ids=list(range(cfg.NCORES)), trace=_trace)
    y = np.concatenate([np.asarray(res.results[c]["y"]).reshape(-1)
                        for c in range(cfg.NCORES)])
    if _trace:
        kernel._last_result = res
    return y.astype(np.float32)



# revision 53
# speedup vs baseline: 1.9530x; 1.9530x over previous
"""DrBC GNN message-passing kernel for 8 Trainium2 NeuronCores.

Strategy (graph/data parallel, per sharding hint):
 - Nodes sharded contiguously across 8 cores (6250/core). Edges partitioned
   by destination core so segment_sum is local.
 - Full bf16 pipeline (fp32 PSUM accumulation): 4x PE throughput over fp32
   and half the DMA/collective bytes; measured rel err 1.25e-2 (< 2e-2).
   fp8 for the table or one-hot weights was host-simulated at 3-5e-2: too
   lossy.
 - Aggregation h_agg^T = sum_e msg[e,:] * onehot(dst_e)*norm_e runs on the
   TensorEngine: edges are bucketed on the host into (dst-window, src-segment)
   chunks of 128; per chunk a [128e x 128d] norm-scaled one-hot tile S^T is
   streamed from DRAM; PSUM accumulates per 128-node window.
 - Messages h[src] are fetched with GPSIMD dma_gather (single-packet calls of
   <=7 chunks on 4 SWDGE queues -- measured fastest; multi-packet ~2x slower,
   8-chunk packets crash the runtime, indirect_dma_start ~8x slower) from a
   bf16 node-major table in DRAM. int16 indices address the table in two
   segments of <=32768 rows.
 - Per layer each core's updated h shard is transposed (PE) to node-major
   rows and redistributed with two AllGathers whose outputs are the two
   gather-table segments (segment A = all cores' rows [0,4096) == the int16
   low segment). The segment-A AllGather is issued mid-layer so most of its
   latency overlaps the tail groups' compute; collectives must issue on the
   Pool engine (verifier) so they serialize with gather descriptor-gen --
   the remaining per-layer cost is ~150us CC + ~215us gather.
 - GRU runs feature-major ([128 feat x 512 nodes] tiles) with the six 128x128
   weight blocks stationary on the PE; gates on the Scalar engine.
 - BatchNorm (training mode, global over nodes): encode-BN folded on the host
   (data-independent); decode-BN via a tiny AllReduce of per-core (sum,sumsq)
   with stats fused into the last layer's aggregation pass.
"""

import os
import sys
from dataclasses import dataclass, field

import numpy as np

sys.path.insert(0, "/opt/trn_rl_repo")

import concourse.bass as bass  # noqa: E402
import concourse.bacc as bacc  # noqa: E402
import concourse.tile as tile  # noqa: E402
from concourse import mybir  # noqa: E402
from concourse import bass_utils  # noqa: E402
from concourse.masks import make_identity  # noqa: E402
from concourse.tile import add_dep_helper  # noqa: E402
from concourse.ap import AP as RawAP  # noqa: E402
from ml_dtypes import bfloat16  # noqa: E402

FP = mybir.dt.float32
BF = mybir.dt.bfloat16
I16 = mybir.dt.int16
AX = mybir.AxisListType
ALU = mybir.AluOpType
ACT = mybir.ActivationFunctionType


@dataclass
class Cfg:
    N: int = 50000
    D: int = 128
    L: int = 5
    NCORES: int = 8
    W: int = 128         # dst window width (= agg matmul free dim)
    GW: int = 4          # windows per group (group = GRU node tile of GW*W)
    BLK: int = 32768     # int16 gather index block size
    EPS: float = 1e-5
    msg_bufs: int = 2
    NQ: int = 4
    QSPLIT: int = 0  # >0: N-way call split; <=0: single-packet calls of <=7 chunks
    MAXSP: int = 7   # chunks per single-packet gather call
    CCEND: bool = False  # emit both table CCs at layer end
    cc_eng: str = "gpsimd"  # engine issuing the table CCs (gpsimd|vector|scalar)
    LREP: int = 1  # timing amplifier: repeat the layer loop
    F32: bool = False  # full-fp32 pipeline (gather is descriptor-bound, so ~free)
    SEG: int = 4096      # rows/core in table segment A (8*SEG == BLK)
    CCSPLIT: bool = True  # two overlapped per-segment AllGathers per layer
    cc_mini: bool = False  # timing probe: tiny CCs in place of table CCs
    TDIRECT: bool = False  # direct table copy: data not visible cross-core
    GIND: bool = False   # gather via indirect_dma_start: ~8x slower, dead end
    # ablation switches (timing experiments only; results become wrong)
    abl_cc: bool = False
    abl_gather: bool = False
    abl_slab: bool = False
    abl_aggmm: bool = False
    abl_gru: bool = False
    abl_wt: bool = False

    @property
    def NL(self):
        return self.N // self.NCORES

    @property
    def NWIN(self):
        return -(-self.NL // self.W)

    @property
    def NLP(self):
        return self.NWIN * self.W

    @property
    def NG(self):
        return -(-self.NWIN // self.GW)


@dataclass
class Plan:
    cfg: Cfg
    nch: np.ndarray          # [NWIN, 2] chunks per (window, block), uniform across cores
    order: list              # list of (w, b) in stream order
    chunk_base: dict         # (w, b) -> first chunk index
    TOTCH: int = 0
    calls: list = field(default_factory=list)   # [g][b] = (ch0, ncc)
    MAXC: int = 0
    sched: list = field(default_factory=list)   # [g] = list of (ch, wl, first, last)
    in_maps: list = field(default_factory=list)


def _win_range(cfg, g):
    w0 = g * cfg.GW
    w1 = min((g + 1) * cfg.GW, cfg.NWIN)
    return range(w0, w1)


def make_plan(cfg: Cfg, edge_index, norm, x, W_enc, bn_e_gamma, bn_e_beta,
              W_ih, W_hh, b_ih, b_hh, W_dec, bn_d_gamma, bn_d_beta, W_dec2):
    src = np.asarray(edge_index[0], dtype=np.int64)
    dst = np.asarray(edge_index[1], dtype=np.int64)
    norm = np.asarray(norm, dtype=np.float32)
    NL, W, NWIN, NCORES = cfg.NL, cfg.W, cfg.NWIN, cfg.NCORES

    core_of = dst // NL
    percore = []
    cnt = np.zeros((NCORES, NWIN, 2), dtype=np.int64)
    for c in range(NCORES):
        sel = core_of == c
        s, dl, nm = src[sel], dst[sel] - c * NL, norm[sel]
        w = dl // W
        if cfg.CCSPLIT and not cfg.GIND:
            # two-segment table layout: tfull = [all cores' rows 0:SEG |
            # all cores' rows SEG:NL]; segment boundary == BLK so the int16
            # b-split coincides with the two AllGather segments.
            assert NCORES * cfg.SEG == cfg.BLK
            sc_ = s // NL
            sr = s % NL
            s = np.where(sr < cfg.SEG, sc_ * cfg.SEG + sr,
                         NCORES * cfg.SEG + sc_ * (NL - cfg.SEG)
                         + (sr - cfg.SEG))
        if cfg.GIND:
            b = np.zeros_like(s)  # int32 indices: one view, no block split
        else:
            b = (s >= cfg.BLK).astype(np.int64)
        np.add.at(cnt[c], (w, b), 1)
        percore.append((s, dl, nm, w, b))

    nch = -(-cnt.max(axis=0) // 128)             # [NWIN, 2]
    nch[:, 0] = np.maximum(nch[:, 0], 1)         # every window has >=1 chunk

    order, chunk_base = [], {}
    acc = 0
    calls = []
    for g in range(cfg.NG):
        calls.append([])
        for b in (0, 1):
            ch0 = acc
            for w in _win_range(cfg, g):
                order.append((w, b))
                chunk_base[(w, b)] = acc
                acc += int(nch[w, b])
            calls[g].append((ch0, acc - ch0))
    TOTCH = acc
    MAXC = max(ncc for g in calls for (_, ncc) in g if ncc > 0)

    # matmul schedule per group: (chunk, window-local, first, last)
    sched = []
    for g in range(cfg.NG):
        items = []
        for b in (0, 1):
            for w in _win_range(cfg, g):
                base = chunk_base[(w, b)]
                for j in range(int(nch[w, b])):
                    items.append([base + j, w - g * cfg.GW, False, False])
        # one accumulation group per PSUM bank: start on globally-first matmul,
        # stop on globally-last (first touch of each element overwrites via
        # the pending-zero / has_written mechanism).
        items[0][2] = True
        items[-1][3] = True
        sched.append([tuple(it) for it in items])

    plan = Plan(cfg=cfg, nch=nch, order=order, chunk_base=chunk_base,
                TOTCH=TOTCH, calls=calls, MAXC=MAXC, sched=sched)

    # ---- per-core input tensors ----
    xdt = np.float32 if cfg.F32 else bfloat16
    D = cfg.D
    W_ih = np.asarray(W_ih, np.float32)
    W_hh = np.asarray(W_hh, np.float32)
    b_ih = np.asarray(b_ih, np.float32)
    b_hh = np.asarray(b_hh, np.float32)
    gru = np.concatenate([
        W_ih[0:D].T, W_ih[D:2 * D].T, W_ih[2 * D:3 * D].T,
        W_hh[0:D].T, W_hh[D:2 * D].T, W_hh[2 * D:3 * D].T,
    ], axis=1).astype(xdt)                                # [128, 6*128]
    gbias = np.stack([
        b_ih[0:D] + b_hh[0:D], b_ih[D:2 * D] + b_hh[D:2 * D],
        b_ih[2 * D:3 * D], b_hh[2 * D:3 * D],
    ], axis=1).astype(np.float32)                          # [128, 4]
    wenc = np.ascontiguousarray(np.asarray(W_enc, np.float32).T)   # [3,128]
    # encode BN stats are data-independent of device state: e0 = x @ W_enc.T
    e0 = np.asarray(x, np.float64) @ np.asarray(W_enc, np.float64).T
    mu_e = e0.mean(0)
    var_e = e0.var(0)
    sc = np.asarray(bn_e_gamma, np.float64) / np.sqrt(var_e + cfg.EPS)
    sh = np.asarray(bn_e_beta, np.float64) - mu_e * sc
    bne = np.stack([sc, sh], axis=1).astype(np.float32)            # [128,2]
    wdec = np.ascontiguousarray(np.asarray(W_dec, np.float32).T).astype(xdt)  # [128,64]
    bnd = np.stack([np.asarray(bn_d_gamma, np.float32),
                    np.asarray(bn_d_beta, np.float32)], axis=1)    # [64,2]
    wdec2 = np.ascontiguousarray(np.asarray(W_dec2, np.float32).T).astype(xdt)  # [64,1]

    x = np.asarray(x, np.float32)
    for c in range(NCORES):
        s, dl, nm, w, b = percore[c]
        idx_arr = np.zeros(TOTCH * 128, np.int32 if cfg.GIND else np.int16)
        sval = np.zeros(TOTCH * 128, np.float32)
        dcol = np.zeros(TOTCH * 128, np.int64)
        # stable bucket fill
        key = w * 2 + b
        ordk = np.argsort(key, kind="stable")
        s, dl, nm, w, b = s[ordk], dl[ordk], nm[ordk], w[ordk], b[ordk]
        ks = key[ordk]
        if len(ks):
            change = np.r_[True, ks[1:] != ks[:-1]]
            run_start = np.flatnonzero(change)
            run_id = np.cumsum(change) - 1
            pos = np.arange(len(s)) - run_start[run_id]
        else:
            pos = np.zeros(0, np.int64)
        cb_lut = np.zeros(NWIN * 2, np.int64)
        for (wi, bi), v in chunk_base.items():
            cb_lut[wi * 2 + bi] = v
        slot = cb_lut[ks] * 128 + pos
        if cfg.GIND:
            idx_arr[slot] = s.astype(np.int32)
        else:
            idx_arr[slot] = (s - b * cfg.BLK).astype(np.int16)
        sval[slot] = nm
        dcol[slot] = dl - w * cfg.W

        S = np.zeros((TOTCH, 128, cfg.W), np.float32)
        ii = np.arange(TOTCH * 128)
        S[ii // 128, ii % 128, dcol] = sval
        stab = np.ascontiguousarray(
            S.transpose(1, 0, 2).reshape(128, TOTCH * cfg.W)).astype(xdt)
        if cfg.GIND:
            # [128, TOTCH]: column ch, partition p = edge slot ch*128+p
            idx_wrapped = np.ascontiguousarray(idx_arr.reshape(TOTCH, 128).T)
        else:
            idx_wrapped = np.ascontiguousarray(np.tile(idx_arr.reshape(TOTCH * 8, 16).T, (8, 1)))

        xT = np.zeros((3, cfg.NLP), np.float32)
        xT[:, :NL] = x[c * NL:(c + 1) * NL].T

        plan.in_maps.append({
            "xT": xT, "idx": idx_wrapped, "stab": stab,
            "wenc": wenc, "gru": gru, "gbias": gbias, "bne": bne,
            "wdec": wdec, "bnd": bnd, "wdec2": wdec2,
        })
    return plan


def build_program(plan: Plan, num_devices: int):
    cfg = plan.cfg
    N, D, NL, NLP, W, GW, NG, L = (cfg.N, cfg.D, cfg.NL, cfg.NLP, cfg.W,
                                   cfg.GW, cfg.NG, cfg.L)
    TOTCH, MAXC = plan.TOTCH, plan.MAXC
    RG = [list(range(num_devices))]

    XD = FP if cfg.F32 else BF
    nc = bacc.Bacc("TRN2", target_bir_lowering=False, debug=False,
                   num_devices=num_devices, num_swdge_queues=cfg.NQ)

    # ---- external tensors ----
    t_xT = nc.dram_tensor("xT", [3, NLP], FP, kind="ExternalInput").ap()
    if cfg.GIND:
        t_idx = nc.dram_tensor("idx", [128, TOTCH], mybir.dt.int32,
                               kind="ExternalInput").ap()
    else:
        t_idx = nc.dram_tensor("idx", [128, TOTCH * 8], I16,
                               kind="ExternalInput").ap()
    t_stab = nc.dram_tensor("stab", [128, TOTCH * W], XD, kind="ExternalInput").ap()
    t_wenc = nc.dram_tensor("wenc", [3, 128], FP, kind="ExternalInput").ap()
    t_gru = nc.dram_tensor("gru", [128, 6 * 128], XD, kind="ExternalInput").ap()
    t_gbias = nc.dram_tensor("gbias", [128, 4], FP, kind="ExternalInput").ap()
    t_bne = nc.dram_tensor("bne", [128, 2], FP, kind="ExternalInput").ap()
    t_wdec = nc.dram_tensor("wdec", [128, 64], XD, kind="ExternalInput").ap()
    t_bnd = nc.dram_tensor("bnd", [64, 2], FP, kind="ExternalInput").ap()
    t_wdec2 = nc.dram_tensor("wdec2", [64, 1], XD, kind="ExternalInput").ap()
    t_y = nc.dram_tensor("y", [NL], FP, kind="ExternalOutput").ap()

    GN = GW * W

    def gwidth(g):
        return min(GN, NLP - g * GN)

    def nodes_slice(g):
        return slice(g * GN, g * GN + gwidth(g))

    if cfg.TDIRECT:
        assert cfg.CCSPLIT
    assert not (cfg.GIND and cfg.CCSPLIT), "GIND needs the single-table view"
    with tile.TileContext(nc) as tc:
        with (
            tc.tile_pool(name="const", bufs=1) as cpool,
            tc.tile_pool(name="state", bufs=1) as spool,
            tc.tile_pool(name="msg", bufs=cfg.msg_bufs) as mpool,
            tc.tile_pool(name="work", bufs=2) as wpool,
            tc.tile_pool(name="psA", bufs=2, space="PSUM") as psA,
            tc.tile_pool(name="psG", bufs=1, space="PSUM") as psG,
            tc.tile_pool(name="psT", bufs=2, space="PSUM") as psT,
            tc.tile_pool(name="dram", bufs=1, space="DRAM") as dpool,
        ):
            # ---------- load constants ----------
            if cfg.GIND:
                idx_sb = cpool.tile([128, TOTCH], mybir.dt.int32)
            else:
                idx_sb = cpool.tile([128, TOTCH * 8], I16)
            nc.sync.dma_start(idx_sb[:], t_idx)
            wenc_sb = cpool.tile([3, 128], FP)
            nc.sync.dma_start(wenc_sb[:], t_wenc)
            gru_sb = cpool.tile([128, 6 * 128], XD)
            nc.sync.dma_start(gru_sb[:], t_gru)
            gbias_sb = cpool.tile([128, 4], FP)
            nc.sync.dma_start(gbias_sb[:], t_gbias)
            bne_sb = cpool.tile([128, 2], FP)
            nc.sync.dma_start(bne_sb[:], t_bne)
            wdec_sb = cpool.tile([128, 64], XD)
            nc.sync.dma_start(wdec_sb[:], t_wdec)
            bnd_sb = cpool.tile([64, 2], FP)
            nc.sync.dma_start(bnd_sb[:], t_bnd)
            wdec2_sb = cpool.tile([64, 1], XD)
            nc.sync.dma_start(wdec2_sb[:], t_wdec2)
            ident_bf = cpool.tile([128, 128], XD)
            make_identity(nc, ident_bf[:])

            def gw(i):  # gru weight block i as lhsT [128,128]
                return gru_sb[:, i * 128:(i + 1) * 128]

            # ---------- state ----------
            hsum = spool.tile([128, NLP], FP)
            prevh = spool.tile([128, NLP], XD)

            # internal DRAM
            TL = L * cfg.LREP
            tin = [dpool.tile([NL, D], XD, name=f"tin{i}") for i in range(TL)]
            if cfg.CCSPLIT:
                # one tensor per AllGather segment: keeps the sim's
                # single-writer check happy and gives exact read deps
                tfullA = [dpool.tile([cfg.BLK, D], XD, addr_space="Shared",
                                     name=f"tfA{i}") for i in range(TL)]
                tfullB = [dpool.tile([N - cfg.BLK, D], XD, addr_space="Shared",
                                     name=f"tfB{i}") for i in range(TL)]
            else:
                tfull = [dpool.tile([N, D], XD, addr_space="Shared",
                                    name=f"tfull{i}") for i in range(TL)]
            bar_i = dpool.tile([1, D], FP, name="bar_i")
            nc.sync.dma_start(bar_i[:, :], t_xT[0:1, 0:D])
            stats_io = [dpool.tile([128, 2], FP, addr_space=("Shared" if i else "Local"),
                                   name=f"stio{i}") for i in range(2)]
            stats2_io = [dpool.tile([64, 2], FP, addr_space=("Shared" if i else "Local"),
                                    name=f"st2io{i}") for i in range(2)]

            # small persistent tiles
            acc_s = spool.tile([128, NG], FP)
            acc_q = spool.tile([128, NG], FP)
            accd_s = spool.tile([64, NG], FP)
            accd_q = spool.tile([64, NG], FP)
            stats_sb = spool.tile([128, 2], FP)
            gstats_sb = spool.tile([128, 2], FP)
            scale_e = spool.tile([128, 1], FP)
            shift_e = spool.tile([128, 1], FP)
            stats2_sb = spool.tile([64, 2], FP)
            gstats2_sb = spool.tile([64, 2], FP)
            scale_d = spool.tile([64, 1], FP)
            shift_d = spool.tile([64, 1], FP)
            t_y2d = t_y.rearrange("(a b) -> a b", a=1)

            def bn_coeffs(gst, gamma_beta, scale_t, shift_t, nelem, P):
                """scale/shift from global (sum,sumsq) in gst [P,2]."""
                mu = wpool.tile([128, 1], FP, tag="bn1")
                ex2 = wpool.tile([128, 1], FP, tag="bn2")
                var = wpool.tile([128, 1], FP, tag="bn3")
                std = wpool.tile([128, 1], FP, tag="bn4")
                nc.vector.tensor_scalar_mul(mu[:P], gst[:, 0:1], 1.0 / nelem)
                nc.vector.tensor_scalar_mul(ex2[:P], gst[:, 1:2], 1.0 / nelem)
                nc.vector.tensor_tensor(var[:P], mu[:P], mu[:P], op=ALU.mult)
                nc.vector.tensor_tensor(var[:P], ex2[:P], var[:P], op=ALU.subtract)
                nc.vector.tensor_scalar_add(var[:P], var[:P], cfg.EPS)
                nc.scalar.activation(std[:P], var[:P], ACT.Sqrt)
                nc.vector.reciprocal(std[:P], std[:P])
                nc.vector.tensor_tensor(scale_t[:], std[:P], gamma_beta[:, 0:1],
                                        op=ALU.mult)
                nc.vector.tensor_tensor(var[:P], mu[:P], scale_t[:], op=ALU.mult)
                nc.vector.tensor_tensor(shift_t[:], gamma_beta[:, 1:2], var[:P],
                                        op=ALU.subtract)

            def allreduce_stats(sb, io_pair, P):
                nc.sync.dma_start(io_pair[0][:], sb[:P, :])
                nc.gpsimd.collective_compute(
                    "AllReduce", ALU.add, replica_groups=RG,
                    ins=[io_pair[0].opt()], outs=[io_pair[1].opt()])
                return io_pair[1]

            # ================= ENCODE =================
            scratch = wpool.tile([128, GN], FP, tag="scr")

            def load_xt(g):
                wd = gwidth(g)
                xt = wpool.tile([3, GN], FP, tag="xt")
                nc.sync.dma_start(xt[:, :wd], t_xT[:, nodes_slice(g)])
                return xt

            def write_table(l, g):
                """transpose prevh group g to node-major rows of tin[l%2]."""
                wd = gwidth(g)
                for k in range(wd // 128 + (1 if wd % 128 else 0)):
                    n0 = g * GN + k * 128
                    rows = min(128, NL - n0)
                    if rows <= 0:
                        continue
                    pst = psT.tile([128, 128], XD, tag="tr")
                    nc.tensor.transpose(pst[:], prevh[:, n0:n0 + 128], ident_bf[:])
                    tt = wpool.tile([128, 128], XD, tag="tt")
                    nc.scalar.activation(tt[:], pst[:], ACT.Copy)
                    nc.sync.dma_start(tin[l][n0:n0 + rows, :], tt[:rows, :])

            barrier = {}

            def emit_cc(l, seg):
                """AllGather one table segment (two-segment shared layout)."""
                if cfg.abl_cc:
                    return
                if cfg.TDIRECT:
                    # each core DMA-copies its shard into the shared segment
                    # at a pid-dependent offset (single writer instruction per
                    # tensor), then a tiny AllGather acts as the cross-core
                    # barrier; gathers take a manual dep on it.
                    if seg == 0:
                        rows, s0, tf = cfg.SEG, 0, tfullA[l]
                    else:
                        rows, s0, tf = NL - cfg.SEG, cfg.SEG, tfullB[l]
                    dsl = tf[0:rows, :]
                    pid = nc.sync.partition_id()
                    dst = RawAP(tensor=dsl.tensor, offset=pid * (rows * D),
                                ap=dsl.ap, dep_tracking_offset=0)
                    cp = nc.sync.dma_start(dst, tin[l][s0:s0 + rows, :])
                    bar_o = dpool.tile([num_devices, D], FP,
                                       addr_space="Shared",
                                       name=f"bo{l}_{seg}")
                    cc = nc.gpsimd.collective_compute(
                        "AllGather", ALU.bypass, replica_groups=RG,
                        ins=[bar_i.opt()], outs=[bar_o.opt()])
                    add_dep_helper(cc.ins, cp.ins, reason="barrier after table copy")
                    barrier[(l, seg)] = cc
                    return
                if cfg.cc_mini:
                    # tiny CC in the same dependency position (timing probe):
                    # in <- one tin row (orders after table writes), out -> one
                    # row of the segment (orders gathers after the CC)
                    s0 = 0 if seg == 0 else cfg.SEG
                    tf = tfullA[l] if seg == 0 else tfullB[l]
                    mbar_i = dpool.tile([1, D], XD, name=f"bi{l}_{seg}")
                    bar_o = dpool.tile([num_devices, D], XD,
                                       addr_space="Shared",
                                       name=f"bo{l}_{seg}")
                    bsb = wpool.tile([1, D], XD, tag="barb")
                    nc.sync.dma_start(bsb[:, :], tin[l][s0:s0 + 1, :])
                    nc.sync.dma_start(mbar_i[:, :], bsb[:, :])
                    nc.gpsimd.collective_compute(
                        "AllGather", ALU.bypass, replica_groups=RG,
                        ins=[mbar_i.opt()], outs=[bar_o.opt()])
                    nc.sync.dma_start(tf[0:1, :], bar_o[0:1, :])
                    return
                cce = {"gpsimd": nc.gpsimd, "vector": nc.vector,
                       "scalar": nc.scalar}[cfg.cc_eng]
                if seg == 0:
                    bass.BassGpSimd.collective_compute(
                        cce, "AllGather", ALU.bypass, replica_groups=RG,
                        ins=[tin[l][0:cfg.SEG, :].opt()],
                        outs=[tfullA[l].opt()])
                else:
                    bass.BassGpSimd.collective_compute(
                        cce, "AllGather", ALU.bypass, replica_groups=RG,
                        ins=[tin[l][cfg.SEG:NL, :].opt()],
                        outs=[tfullB[l].opt()])

            for g in range(NG):
                wd = gwidth(g)
                sl = nodes_slice(g)
                xt = load_xt(g)
                ps = psA.tile([128, GN], FP, tag="agg")
                nc.tensor.matmul(ps[:, :wd], wenc_sb[:], xt[:, :wd],
                                 start=True, stop=True)
                nc.scalar.activation(prevh[:, sl], ps[:, :wd], ACT.Relu,
                                     bias=bne_sb[:, 1:2], scale=bne_sb[:, 0:1])
                nc.scalar.activation(hsum[:, sl], ps[:, :wd], ACT.Relu,
                                     bias=bne_sb[:, 1:2], scale=bne_sb[:, 0:1])
                if not cfg.abl_wt:
                    write_table(0, g)
                if cfg.CCSPLIT and g == cfg.SEG // GN - 1:
                    emit_cc(0, 0)
            if cfg.CCSPLIT:
                emit_cc(0, 1)

            scratch2 = wpool.tile([64, GN], FP, tag="scr2")
            # ================= LAYERS =================
            # queue must track Tile's DMASW lane round-robin (lane = k % 8,
            # queue = k % NQ with NQ | 8) -> single global SWDGE-DMA counter.
            qctr = 0
            for l in range(TL):
                last = l == TL - 1
                if not cfg.CCSPLIT and not cfg.abl_cc:
                    nc.gpsimd.collective_compute(
                        "AllGather", ALU.bypass, replica_groups=RG,
                        ins=[tin[l].opt()], outs=[tfull[l].opt()])
                if cfg.CCSPLIT:
                    tblA, tblB = tfullA[l][:, :], tfullB[l][:, :]
                else:
                    tblA, tblB = tfull[l][:, :], tfull[l][cfg.BLK:, :]
                for g in range(NG):
                    wd = gwidth(g)
                    sl = nodes_slice(g)
                    ps = psA.tile([128, GN], FP, tag="agg")
                    msgs = {}
                    for b in (0, 1):
                        ch0, ncc = plan.calls[g][b]
                        if ncc == 0:
                            continue
                        m = mpool.tile([128, MAXC, 128], XD, tag="msg")
                        if cfg.abl_gather:
                            # keep the tile allocated (timing ablation only)
                            nc.sync.dma_start(m[:, 0, :], t_stab[:, 0:128])
                        src_view = tblB if b else tblA
                        if cfg.GIND:
                            if not cfg.abl_gather:
                                nc.gpsimd.indirect_dma_start(
                                    out=m[:, 0:ncc, :], out_offset=None,
                                    in_=src_view,
                                    in_offset=bass.IndirectOffsetOnAxis(
                                        ap=idx_sb[:, ch0:ch0 + ncc], axis=0))
                        else:
                            if cfg.QSPLIT > 0:
                                nsplit = min(cfg.QSPLIT, ncc)
                                sp = False
                            else:  # QSPLIT<=0: single-packet calls
                                nsplit = -(-ncc // cfg.MAXSP)
                                sp = True
                            per = -(-ncc // nsplit)
                            for si in range(nsplit):
                                c0 = si * per
                                c1 = min(ncc, c0 + per)
                                if c1 <= c0:
                                    continue
                                if not cfg.abl_gather:
                                    gi = nc.gpsimd.dma_gather(
                                        m[:, c0:c1, :], src_view,
                                        idx_sb[:, (ch0 + c0) * 8:(ch0 + c1) * 8],
                                        (c1 - c0) * 128, (c1 - c0) * 128, 128,
                                        elem_step=D, single_packet=sp,
                                        queue_num=qctr % cfg.NQ)
                                    qctr += 1
                                    bb = barrier.get((l, b))
                                    if bb is not None:
                                        add_dep_helper(
                                            gi.ins, bb.ins,
                                            reason="gather after table barrier")
                        slab = mpool.tile([128, MAXC * W], XD, tag="slab")
                        if cfg.abl_slab:
                            nc.sync.dma_start(slab[:, 0:W], t_stab[:, 0:W])
                        else:
                            nc.sync.dma_start(slab[:, :ncc * W],
                                              t_stab[:, ch0 * W:(ch0 + ncc) * W])
                        msgs[b] = (m, slab, ch0)
                    if (cfg.CCSPLIT and not cfg.CCEND and not last
                            and g == cfg.SEG // GN):
                        emit_cc(l + 1, 0)
                    b1_start = plan.calls[g][1][0]
                    b1_n = plan.calls[g][1][1]
                    if cfg.abl_aggmm:
                        m, slab, ch0 = msgs[0]
                        for wl in range(-(-wd // W)):
                            nc.tensor.matmul(
                                ps[:, wl * W:(wl + 1) * W], m[:, 0, :],
                                slab[:, 0:W], start=True, stop=True,
                                skip_group_check=True)
                    else:
                        for (ch, wl, first, lastc) in plan.sched[g]:
                            b = 1 if (b1_n > 0 and ch >= b1_start) else 0
                            m, slab, ch0 = msgs[b]
                            nc.tensor.matmul(
                                ps[:, wl * W:(wl + 1) * W], m[:, ch - ch0, :],
                                slab[:, (ch - ch0) * W:(ch - ch0 + 1) * W],
                                start=first, stop=lastc, skip_group_check=True)
                    # h_sum += h_agg
                    nc.vector.tensor_tensor(hsum[:, sl], hsum[:, sl], ps[:, :wd],
                                            op=ALU.add)
                    if last:
                        # fused decode stats (pass 1) for this group
                        lim = min(wd, max(0, NL - g * GN))
                        zb = wpool.tile([128, GN], XD, tag="zb")
                        nc.scalar.activation(zb[:, :wd], hsum[:, sl], ACT.Copy,
                                             scale=1.0 / (L + 1))
                        ps_y = psG.tile([128, GN], FP, tag="ps_r")
                        nc.tensor.matmul(ps_y[:64, :wd], wdec_sb[:], zb[:, :wd],
                                         start=True, stop=True)
                        nc.scalar.activation(scratch2[:, :lim], ps_y[:64, :lim],
                                             ACT.Copy, accum_out=accd_s[:, g:g + 1])
                        nc.scalar.activation(scratch2[:, :lim], ps_y[:64, :lim],
                                             ACT.Square, accum_out=accd_q[:, g:g + 1])
                        continue
                    if cfg.abl_gru:
                        if not cfg.abl_wt:
                            write_table(l + 1, g)
                        continue
                    aggbf = wpool.tile([128, GN], XD, tag="aggbf")
                    nc.scalar.activation(aggbf[:, :wd], ps[:, :wd], ACT.Copy)
                    # GRU
                    ph = prevh[:, sl]
                    ab = aggbf[:, :wd]
                    ps_r = psG.tile([128, GN], FP, tag="ps_r")
                    ps_z = psG.tile([128, GN], FP, tag="ps_z")
                    ps_i = psG.tile([128, GN], FP, tag="ps_i")
                    ps_h = psG.tile([128, GN], FP, tag="ps_h")
                    nc.tensor.matmul(ps_r[:, :wd], gw(0), ab, start=True, stop=False)
                    nc.tensor.matmul(ps_r[:, :wd], gw(3), ph, start=False, stop=True)
                    nc.tensor.matmul(ps_z[:, :wd], gw(1), ab, start=True, stop=False)
                    nc.tensor.matmul(ps_z[:, :wd], gw(4), ph, start=False, stop=True)
                    nc.tensor.matmul(ps_i[:, :wd], gw(2), ab, start=True, stop=True)
                    nc.tensor.matmul(ps_h[:, :wd], gw(5), ph, start=True, stop=True)
                    r_t = wpool.tile([128, GN], XD, tag="r_t")
                    z_t = wpool.tile([128, GN], XD, tag="z_t")
                    hn_t = wpool.tile([128, GN], XD, tag="hn_t")
                    t2 = wpool.tile([128, GN], FP, tag="t2")
                    n_t = wpool.tile([128, GN], XD, tag="n_t")
                    d_t = wpool.tile([128, GN], XD, tag="d_t")
                    nc.scalar.activation(r_t[:, :wd], ps_r[:, :wd], ACT.Sigmoid,
                                         bias=gbias_sb[:, 0:1])
                    nc.scalar.activation(z_t[:, :wd], ps_z[:, :wd], ACT.Sigmoid,
                                         bias=gbias_sb[:, 1:2])
                    nc.scalar.activation(hn_t[:, :wd], ps_h[:, :wd], ACT.Identity,
                                         bias=gbias_sb[:, 3:4])
                    nc.vector.tensor_tensor(t2[:, :wd], r_t[:, :wd], hn_t[:, :wd],
                                            op=ALU.mult)
                    nc.vector.tensor_tensor(t2[:, :wd], t2[:, :wd], ps_i[:, :wd],
                                            op=ALU.add)
                    nc.scalar.activation(n_t[:, :wd], t2[:, :wd], ACT.Tanh,
                                         bias=gbias_sb[:, 2:3])
                    nc.vector.tensor_tensor(d_t[:, :wd], ph, n_t[:, :wd],
                                            op=ALU.subtract)
                    nc.vector.tensor_tensor(d_t[:, :wd], z_t[:, :wd], d_t[:, :wd],
                                            op=ALU.mult)
                    nc.vector.tensor_tensor(ph, n_t[:, :wd], d_t[:, :wd],
                                            op=ALU.add)
                    if not cfg.abl_wt:
                        write_table(l + 1, g)
                if cfg.CCSPLIT and not last:
                    if cfg.CCEND:
                        emit_cc(l + 1, 0)
                    emit_cc(l + 1, 1)

            # ================= DECODE =================
            inv6 = 1.0 / (L + 1)
            nc.vector.reduce_sum(stats2_sb[:, 0:1], accd_s[:], axis=AX.X)
            nc.vector.reduce_sum(stats2_sb[:, 1:2], accd_q[:], axis=AX.X)
            gst2 = allreduce_stats(stats2_sb, stats2_io, 64)
            nc.sync.dma_start(gstats2_sb[:], gst2[:])
            bn_coeffs(gstats2_sb, bnd_sb, scale_d, shift_d, N, 64)

            for g in range(NG):
                wd = gwidth(g)
                sl = nodes_slice(g)
                zb = wpool.tile([128, GN], XD, tag="zb")
                nc.scalar.activation(zb[:, :wd], hsum[:, sl], ACT.Copy, scale=inv6)
                ps = psA.tile([128, GN], FP, tag="agg")
                nc.tensor.matmul(ps[:64, :wd], wdec_sb[:], zb[:, :wd],
                                 start=True, stop=True)
                y2 = wpool.tile([64, GN], XD, tag="y2")
                nc.scalar.activation(y2[:, :wd], ps[:64, :wd], ACT.Relu,
                                     bias=shift_d[:], scale=scale_d[:])
                psf = psA.tile([1, GN], FP, tag="agg")
                nc.tensor.matmul(psf[:, :wd], wdec2_sb[:], y2[:, :wd],
                                 start=True, stop=True)
                lim = min(wd, max(0, NL - g * GN))
                yt = wpool.tile([1, GN], FP, tag="yt")
                nc.vector.tensor_copy(yt[:, :wd], psf[:, :wd])
                nc.sync.dma_start(t_y2d[0:1, g * GN:g * GN + lim], yt[0:1, :lim])

    nc.compile()
    return nc


_CACHE = {}


def _get_program(plan, num_devices):
    key = (plan.TOTCH, plan.MAXC, repr(plan.cfg))
    if key not in _CACHE:
        _CACHE[key] = build_program(plan, num_devices)
    return _CACHE[key]


def kernel(x, edge_index, norm, W_enc, bn_e_gamma, bn_e_beta, W_ih, W_hh,
           b_ih, b_hh, W_dec, bn_d_gamma, bn_d_beta, W_dec2, _trace=False):
    cfg = Cfg()
    plan = make_plan(cfg, edge_index, norm, x, W_enc, bn_e_gamma, bn_e_beta,
                     W_ih, W_hh, b_ih, b_hh, W_dec, bn_d_gamma, bn_d_beta, W_dec2)
    nc = _get_program(plan, cfg.NCORES)
    res = bass_utils.run_bass_kernel_spmd(
        nc, plan.in_maps, core_ids=list(range(cfg.NCORES)), trace=_trace)
    y = np.concatenate([np.asarray(res.results[c]["y"]).reshape(-1)
                        for c in range(cfg.NCORES)])
    if _trace:
        kernel._last_result = res
    return y.astype(np.float32)

